# revision 6
# baseline (speedup 1.0000x reference)
"""Trainium2 Bass kernel for nn_InvLocalPatOrientConvolution.

Math: y[b,o,i] = sum_c x[b,c,i] w[o,c,i]
      resp[b,o,a,beta,g] = sum_i y[b,o,i] * (ca1[m_i,a] d1[beta,i] cg1[n_i,g]
                                           + ca2[m_i,a] d2[beta,i] cg2[n_i,g])
      out[b,o] = max over (a,beta,g)

Device algorithm (exact reassociation, "mn-binning"):
  A1[i,a] = ca1[m_idx[i],a] etc. are rank-11 gathers, so with bins
  mn(i) = m_idx[i]*11 + n_idx[i]:
    resp_beta[bo, (a,g)] = sum_mn C1[mn,bo] P1[mn,(a,g)] + C2[mn,bo] P2[mn,(a,g)]
    C_t[mn,bo] = sum_{i in bin mn} d_t[beta,i] y_T[i,bo]   (PE matmul, stationary
                 Ed[i,mn] = d_t[beta,i] * onehot(mn(i)), host-precomputed)
    P_t[(m,n),(a,g)] = ca_t[m,a] * cg_t[n,g]               (host-precomputed)
  Stage-2 accumulates both terms into one PSUM bank, DVE max-reduces.

Sharding: beta axis (32) split 4-per-core across 8 cores; host maxes partials.
Matmul operands are bitcast to float32r (single-pass fp32 PE mode, 4x faster
than fp32 for moving-dim >= 256); accumulation stays fp32 in PSUM.
"""

import numpy as np

B, CIN, COUT, TOTAL, K = 8, 16, 32, 286, 32
MM, MN, BO, IPAD, NCORES = 11, 121, 256, 384, 8

_CACHE = {}


def _build_bass():
    if "nc" in _CACHE:
        return _CACHE["nc"]
    import concourse.bacc as bacc
    import concourse.mybir as mybir
    import concourse.tile as tile

    fp32 = mybir.dt.float32
    f32r = mybir.dt.float32r
    nc = bacc.Bacc("TRN2", target_bir_lowering=False, debug=False,
                   num_devices=NCORES)
    yT = nc.dram_tensor("yT", [IPAD, BO], f32r, kind="ExternalInput")
    eh = nc.dram_tensor("eh", [IPAD, MN], fp32, kind="ExternalInput")
    dT = nc.dram_tensor("dT", [IPAD, 8], fp32, kind="ExternalInput")
    p1 = nc.dram_tensor("p1", [MN, 1024], f32r, kind="ExternalInput")
    p2 = nc.dram_tensor("p2", [MN, 1024], f32r, kind="ExternalInput")
    out = nc.dram_tensor("out", [128, 2], fp32, kind="ExternalOutput")

    with tile.TileContext(nc) as tc:
        with tc.tile_pool(name="const", bufs=1) as cp, \
             tc.tile_pool(name="cps", bufs=1, space="PSUM") as cpsp, \
             tc.tile_pool(name="rps", bufs=2, space="PSUM") as rpsp:
            yT_sb = cp.tile([128, 3, BO], f32r)
            eh_sb = cp.tile([128, 3, MN], fp32)
            dT_sb = cp.tile([128, 3, 8], fp32)
            yT_r = yT.ap().rearrange("(c p) n -> p c n", p=128)
            eh_r = eh.ap().rearrange("(c p) n -> p c n", p=128)
            nc.sync.dma_start(out=dT_sb[:],
                              in_=dT.ap().rearrange("(c p) n -> p c n", p=128))
            # chunked loads so stage-1 starts as soon as chunk 0 lands
            for c in range(3):
                nc.sync.dma_start(out=eh_sb[:, c, :], in_=eh_r[:, c, :])
                nc.sync.dma_start(out=yT_sb[:, c, :], in_=yT_r[:, c, :])
            p1_sb = cp.tile([128, 1024], f32r)
            p2_sb = cp.tile([128, 1024], f32r)
            nc.sync.dma_start(out=p1_sb[:MN, :], in_=p1.ap())
            nc.sync.dma_start(out=p2_sb[:MN, :], in_=p2.ap())

            # Stage 1: C_u[mn, bo] = sum_i Ed_u[i, mn] yT[i, bo];  u = t*4 + j
            # Ed_u[i, mn] = eh[i, mn] * dT[i, u] built on ScalarE (ACT
            # per-partition scale), rounding fp32 -> f32r as walrus requires.
            # u ordered in (j, j+4) pairs so stage-2 for j can start early
            Copy = mybir.ActivationFunctionType.Copy
            uorder = [0, 4, 1, 5, 2, 6, 3, 7]
            ed_sb = cp.tile([128, 3, 8 * MN], f32r)
            c_ps = cpsp.tile([128, 8, BO], fp32)
            for u in uorder:
                for c in range(3):
                    nc.scalar.activation(ed_sb[:, c, u * MN:(u + 1) * MN],
                                         eh_sb[:, c, :], Copy,
                                         scale=dT_sb[:, c, u:u + 1])
                    nc.tensor.matmul(c_ps[:MN, u, :],
                                     ed_sb[:, c, u * MN:(u + 1) * MN],
                                     yT_sb[:, c, :],
                                     start=(c == 0), stop=(c == 2))
            c_sb = cp.tile([128, 8, BO], f32r)
            for u in uorder:
                nc.scalar.copy(c_sb[:MN, u, :], c_ps[:MN, u, :])

            # Stage 2+3: resp[bo, ag] accumulated over the two terms, then max
            mx = cp.tile([128, 8], fp32)
            for j in range(4):
                for boc in range(2):
                    r_ps = rpsp.tile([128, 1024], fp32)
                    for t in range(2):
                        u = t * 4 + j
                        psb = p1_sb if t == 0 else p2_sb
                        lhsT = c_sb[:MN, u, boc * 128:(boc + 1) * 128]
                        for agc in range(2):
                            nc.tensor.matmul(
                                r_ps[:, agc * 512:(agc + 1) * 512], lhsT,
                                psb[:MN, agc * 512:(agc + 1) * 512],
                                start=(t == 0), stop=(t == 1))
                    idx = boc * 4 + j
                    nc.vector.reduce_max(mx[:, idx:idx + 1], r_ps[:],
                                         mybir.AxisListType.X)
            out_sb = cp.tile([128, 2], fp32)
            for boc in range(2):
                nc.vector.reduce_max(out_sb[:, boc:boc + 1],
                                     mx[:, boc * 4:(boc + 1) * 4],
                                     mybir.AxisListType.X)
            nc.sync.dma_start(out=out.ap(), in_=out_sb[:])

    nc.compile()
    _CACHE["nc"] = nc
    return nc


def _prep_inputs(x, weight, d1, d2, ca1, cg1, ca2, cg2, m_idx, n_idx):
    f = lambda a: np.asarray(a, np.float32)
    x, w = f(x), f(weight)
    d1, d2, ca1, cg1, ca2, cg2 = f(d1), f(d2), f(ca1), f(cg1), f(ca2), f(cg2)
    m_idx = np.asarray(m_idx, np.int64)
    n_idx = np.asarray(n_idx, np.int64)

    y = np.einsum("bci,oci->boi", x, w).astype(np.float32)  # [B, COUT, TOTAL]
    yT = np.zeros([IPAD, BO], np.float32)
    yT[:TOTAL, :] = y.transpose(2, 0, 1).reshape(TOTAL, BO)  # bo = b*32+o

    eh = np.zeros([IPAD, MN], np.float32)
    eh[np.arange(TOTAL), m_idx * MM + n_idx] = 1.0

    P1 = (ca1[:, None, :, None] * cg1[None, :, None, :]).reshape(MN, 1024)
    P2 = (ca2[:, None, :, None] * cg2[None, :, None, :]).reshape(MN, 1024)

    shared = {"yT": yT, "eh": eh,
              "p1": np.ascontiguousarray(P1, dtype=np.float32),
              "p2": np.ascontiguousarray(P2, dtype=np.float32)}
    in_maps = []
    for k in range(NCORES):
        dt_ = np.zeros([IPAD, 8], np.float32)
        dt_[:TOTAL, 0:4] = d1[4 * k:4 * k + 4, :].T
        dt_[:TOTAL, 4:8] = d2[4 * k:4 * k + 4, :].T
        in_maps.append({**shared, "dT": dt_})
    return in_maps


def kernel(x, weight, d1, d2, ca1, cg1, ca2, cg2, m_idx, n_idx,
           _trace=False, _tmpdir=None):
    from concourse.bass_utils import run_bass_kernel_spmd

    nc = _build_bass()
    in_maps = _prep_inputs(x, weight, d1, d2, ca1, cg1, ca2, cg2,
                           m_idx, n_idx)
    res = run_bass_kernel_spmd(nc, in_maps, core_ids=list(range(NCORES)),
                               trace=_trace, tmpdir=_tmpdir,
                               trace_cores=list(range(NCORES)) if _trace else None,
                               stitch_traces=False)
    kernel.last_results = res
    m = np.maximum.reduce([r["out"] for r in res.results])  # [128, 2]
    return m.T.reshape(BO).reshape(B, COUT).astype(np.float32)


# revision 7
# speedup vs baseline: 1.0707x; 1.0707x over previous
"""Trainium2 Bass kernel for nn_InvLocalPatOrientConvolution.

Math: y[b,o,i] = sum_c x[b,c,i] w[o,c,i]
      resp[b,o,a,beta,g] = sum_i y[b,o,i] * (ca1[m_i,a] d1[beta,i] cg1[n_i,g]
                                           + ca2[m_i,a] d2[beta,i] cg2[n_i,g])
      out[b,o] = max over (a,beta,g)

Device algorithm (exact reassociation, "mn-binning"):
  A1[i,a] = ca1[m_idx[i],a] etc. are rank-11 gathers, so with bins
  mn(i) = m_idx[i]*11 + n_idx[i]:
    resp_beta[bo, (a,g)] = sum_mn C1[mn,bo] P1[mn,(a,g)] + C2[mn,bo] P2[mn,(a,g)]
    C_t[mn,bo] = sum_{i in bin mn} d_t[beta,i] y_T[i,bo]   (PE matmul, stationary
                 Ed[i,mn] = d_t[beta,i] * onehot(mn(i)), built on DVE)
    P_t[(m,n),(a,g)] = ca_t[m,a] * cg_t[n,g]               (host-precomputed)
  Stage-2 accumulates both terms into one PSUM bank, DVE max-reduces.

Sharding: beta axis (32) split 4-per-core across 8 cores; host maxes partials.
Matmul operands are float32r (single-pass fp32 PE mode, 4x faster than fp32
for moving-dim >= 256); accumulation stays fp32 in PSUM.
"""

import numpy as np

B, CIN, COUT, TOTAL, K = 8, 16, 32, 286, 32
MM, MN, BO, IPAD, NCORES = 11, 121, 256, 384, 8
UW = MN + 8 + BO  # 385 cols: [eh | dT | yT]

_CACHE = {}


def _build_bass():
    if "nc" in _CACHE:
        return _CACHE["nc"]
    import concourse.bacc as bacc
    import concourse.mybir as mybir
    import concourse.tile as tile

    fp32 = mybir.dt.float32
    f32r = mybir.dt.float32r
    nc = bacc.Bacc("TRN2", target_bir_lowering=False, debug=False,
                   num_devices=NCORES)
    uu = nc.dram_tensor("uu", [IPAD, UW], f32r, kind="ExternalInput")
    p12 = nc.dram_tensor("p12", [MN, 2048], f32r, kind="ExternalInput")
    out = nc.dram_tensor("out", [128, 2], fp32, kind="ExternalOutput")

    with tile.TileContext(nc) as tc:
        with tc.tile_pool(name="const", bufs=1) as cp, \
             tc.tile_pool(name="cps", bufs=1, space="PSUM") as cpsp, \
             tc.tile_pool(name="rps", bufs=2, space="PSUM") as rpsp:
            u_sb = cp.tile([128, 3, UW], f32r)
            nc.sync.dma_start(out=u_sb[:],
                              in_=uu.ap().rearrange("(c p) n -> p c n", p=128))
            p_sb = cp.tile([128, 2048], f32r)
            nc.sync.dma_start(out=p_sb[:MN, :], in_=p12.ap())

            # Ed[i, (u, mn)] = dT[i, u] * eh[i, mn], one broadcast TT per chunk
            mult = mybir.AluOpType.mult
            ed_sb = cp.tile([128, 3, 8, MN], f32r)
            for c in range(3):
                nc.vector.tensor_tensor(
                    ed_sb[:, c], u_sb[:, c, MN:MN + 8, None].to_broadcast([128, 8, MN]),
                    u_sb[:, c, None, 0:MN].to_broadcast([128, 8, MN]), mult)

            # Stage 1: C_u[mn, bo] = sum_i Ed_u[i, mn] yT[i, bo];  u = t*4 + j
            # c outer keeps PE streaming; start=True only for the first
            # accumulation group in each PSUM bank (banks hold u pairs).
            uorder = [0, 4, 1, 5, 2, 6, 3, 7]
            c_ps = cpsp.tile([128, 8, BO], fp32)
            for c in range(3):
                for u in uorder:
                    nc.tensor.matmul(c_ps[:MN, u, :], ed_sb[:, c, u, :],
                                     u_sb[:, c, MN + 8:UW],
                                     start=(c == 0 and u in (0, 2, 4, 6)),
                                     stop=(c == 2), skip_group_check=True)
            c_sb = cp.tile([128, 8, BO], f32r)
            nc.scalar.copy(c_sb[:MN], c_ps[:MN])

            # Stage 2+3: resp[bo, ag] accumulated over the two terms, then max
            mx = cp.tile([128, 8], fp32)
            for j in range(4):
                for boc in range(2):
                    r_ps = rpsp.tile([128, 1024], fp32)
                    for t in range(2):
                        u = t * 4 + j
                        lhsT = c_sb[:MN, u, boc * 128:(boc + 1) * 128]
                        for agc in range(2):
                            nc.tensor.matmul(
                                r_ps[:, agc * 512:(agc + 1) * 512], lhsT,
                                p_sb[:MN, t * 1024 + agc * 512:
                                     t * 1024 + (agc + 1) * 512],
                                start=(t == 0), stop=(t == 1))
                    idx = boc * 4 + j
                    nc.vector.reduce_max(mx[:, idx:idx + 1], r_ps[:],
                                         mybir.AxisListType.X)
            out_sb = cp.tile([128, 2], fp32)
            for boc in range(2):
                nc.vector.reduce_max(out_sb[:, boc:boc + 1],
                                     mx[:, boc * 4:(boc + 1) * 4],
                                     mybir.AxisListType.X)
            nc.sync.dma_start(out=out.ap(), in_=out_sb[:])

    nc.compile()
    _CACHE["nc"] = nc
    return nc


def _prep_inputs(x, weight, d1, d2, ca1, cg1, ca2, cg2, m_idx, n_idx):
    f = lambda a: np.asarray(a, np.float32)
    x, w = f(x), f(weight)
    d1, d2, ca1, cg1, ca2, cg2 = f(d1), f(d2), f(ca1), f(cg1), f(ca2), f(cg2)
    m_idx = np.asarray(m_idx, np.int64)
    n_idx = np.asarray(n_idx, np.int64)

    y = np.einsum("bci,oci->boi", x, w).astype(np.float32)  # [B, COUT, TOTAL]

    eh = np.zeros([IPAD, MN], np.float32)
    eh[np.arange(TOTAL), m_idx * MM + n_idx] = 1.0

    P1 = (ca1[:, None, :, None] * cg1[None, :, None, :]).reshape(MN, 1024)
    P2 = (ca2[:, None, :, None] * cg2[None, :, None, :]).reshape(MN, 1024)
    p12 = np.ascontiguousarray(np.concatenate([P1, P2], 1), dtype=np.float32)

    U0 = np.zeros([IPAD, UW], np.float32)
    U0[:, 0:MN] = eh
    U0[:TOTAL, MN + 8:UW] = y.transpose(2, 0, 1).reshape(TOTAL, BO)  # bo=b*32+o

    in_maps = []
    for k in range(NCORES):
        Uk = U0.copy()
        Uk[:TOTAL, MN:MN + 4] = d1[4 * k:4 * k + 4, :].T
        Uk[:TOTAL, MN + 4:MN + 8] = d2[4 * k:4 * k + 4, :].T
        in_maps.append({"uu": Uk, "p12": p12})
    return in_maps


def kernel(x, weight, d1, d2, ca1, cg1, ca2, cg2, m_idx, n_idx,
           _trace=False, _tmpdir=None):
    from concourse.bass_utils import run_bass_kernel_spmd

    nc = _build_bass()
    in_maps = _prep_inputs(x, weight, d1, d2, ca1, cg1, ca2, cg2,
                           m_idx, n_idx)
    res = run_bass_kernel_spmd(nc, in_maps, core_ids=list(range(NCORES)),
                               trace=_trace, tmpdir=_tmpdir,
                               trace_cores=list(range(NCORES)) if _trace else None,
                               stitch_traces=False)
    kernel.last_results = res
    m = np.maximum.reduce([r["out"] for r in res.results])  # [128, 2]
    return m.T.reshape(BO).reshape(B, COUT).astype(np.float32)


# revision 8
# speedup vs baseline: 1.0999x; 1.0273x over previous
"""Trainium2 Bass kernel for nn_InvLocalPatOrientConvolution.

Math: y[b,o,i] = sum_c x[b,c,i] w[o,c,i]
      resp[b,o,a,beta,g] = sum_i y[b,o,i] * (ca1[m_i,a] d1[beta,i] cg1[n_i,g]
                                           + ca2[m_i,a] d2[beta,i] cg2[n_i,g])
      out[b,o] = max over (a,beta,g)

Device algorithm (exact reassociation, "mn-binning"):
  A1[i,a] = ca1[m_idx[i],a] etc. are rank-11 gathers, so with bins
  mn(i) = m_idx[i]*11 + n_idx[i]:
    resp_beta[bo, (a,g)] = sum_mn C1[mn,bo] P1[mn,(a,g)] + C2[mn,bo] P2[mn,(a,g)]
    C_t[mn,bo] = sum_{i in bin mn} d_t[beta,i] y_T[i,bo]   (PE matmul, stationary
                 Ed[i,mn] = d_t[beta,i] * onehot(mn(i)), built on DVE)
    P_t[(m,n),(a,g)] = ca_t[m,a] * cg_t[n,g]               (host-precomputed)
  Stage-2 accumulates both terms into one PSUM bank, DVE max-reduces.

Sharding: beta axis (32) split 4-per-core across 8 cores; host maxes partials.
Matmul operands are float32r (single-pass fp32 PE mode, 4x faster than fp32
for moving-dim >= 256); accumulation stays fp32 in PSUM.
"""

import numpy as np

B, CIN, COUT, TOTAL, K = 8, 16, 32, 286, 32
MM, MN, BO, IPAD, NCORES = 11, 121, 256, 384, 8
UW = MN + 8 + BO  # 385 cols: [eh | dT | yT]

_CACHE = {}


def _build_bass():
    if "nc" in _CACHE:
        return _CACHE["nc"]
    import concourse.bacc as bacc
    import concourse.mybir as mybir
    import concourse.tile as tile

    fp32 = mybir.dt.float32
    f32r = mybir.dt.float32r
    nc = bacc.Bacc("TRN2", target_bir_lowering=False, debug=False,
                   num_devices=NCORES)
    uu = nc.dram_tensor("uu", [IPAD, UW], f32r, kind="ExternalInput")
    p12 = nc.dram_tensor("p12", [MN, 2048], f32r, kind="ExternalInput")
    out = nc.dram_tensor("out", [128, 2], fp32, kind="ExternalOutput")

    with tile.TileContext(nc) as tc:
        with tc.tile_pool(name="const", bufs=1) as cp, \
             tc.tile_pool(name="cps", bufs=1, space="PSUM") as cpsp, \
             tc.tile_pool(name="rps", bufs=2, space="PSUM") as rpsp:
            u_sb = cp.tile([128, 3, UW], f32r)
            uu_r = uu.ap().rearrange("(c p) n -> p c n", p=128)
            for c in range(3):
                nc.sync.dma_start(out=u_sb[:, c], in_=uu_r[:, c])
            p_sb = cp.tile([128, 2048], f32r)
            nc.sync.dma_start(out=p_sb[:MN, :], in_=p12.ap())

            # Ed[i, (u, mn)] = dT[i, u] * eh[i, mn], one broadcast TT per chunk
            mult = mybir.AluOpType.mult
            ed_sb = cp.tile([128, 3, 8, MN], f32r)
            for c in range(3):
                nc.vector.tensor_tensor(
                    ed_sb[:, c], u_sb[:, c, MN:MN + 8, None].to_broadcast([128, 8, MN]),
                    u_sb[:, c, None, 0:MN].to_broadcast([128, 8, MN]), mult)

            # Stage 1: C_u[mn, bo] = sum_i Ed_u[i, mn] yT[i, bo];  u = t*4 + j
            # c outer keeps PE streaming; start=True only for the first
            # accumulation group in each PSUM bank (banks hold u pairs).
            uorder = [0, 4, 1, 5, 2, 6, 3, 7]
            c_ps = cpsp.tile([128, 8, BO], fp32)
            for c in range(3):
                for u in uorder:
                    nc.tensor.matmul(c_ps[:MN, u, :], ed_sb[:, c, u, :],
                                     u_sb[:, c, MN + 8:UW],
                                     start=(c == 0 and u in (0, 2, 4, 6)),
                                     stop=(c == 2), skip_group_check=True)
            c_sb = cp.tile([128, 8, BO], f32r)

            # Stage 2+3: resp[bo, ag] accumulated over the two terms, then max
            mx = cp.tile([128, 8], fp32)
            for j in range(4):
                nc.scalar.copy(c_sb[:MN, j], c_ps[:MN, j])
                nc.scalar.copy(c_sb[:MN, j + 4], c_ps[:MN, j + 4])
                for boc in range(2):
                    r_ps = rpsp.tile([128, 1024], fp32)
                    for t in range(2):
                        u = t * 4 + j
                        lhsT = c_sb[:MN, u, boc * 128:(boc + 1) * 128]
                        for agc in range(2):
                            nc.tensor.matmul(
                                r_ps[:, agc * 512:(agc + 1) * 512], lhsT,
                                p_sb[:MN, t * 1024 + agc * 512:
                                     t * 1024 + (agc + 1) * 512],
                                start=(t == 0), stop=(t == 1))
                    idx = boc * 4 + j
                    nc.vector.reduce_max(mx[:, idx:idx + 1], r_ps[:],
                                         mybir.AxisListType.X)
            out_sb = cp.tile([128, 2], fp32)
            for boc in range(2):
                nc.vector.reduce_max(out_sb[:, boc:boc + 1],
                                     mx[:, boc * 4:(boc + 1) * 4],
                                     mybir.AxisListType.X)
            nc.sync.dma_start(out=out.ap(), in_=out_sb[:])

    nc.compile()
    _CACHE["nc"] = nc
    return nc


def _prep_inputs(x, weight, d1, d2, ca1, cg1, ca2, cg2, m_idx, n_idx):
    f = lambda a: np.asarray(a, np.float32)
    x, w = f(x), f(weight)
    d1, d2, ca1, cg1, ca2, cg2 = f(d1), f(d2), f(ca1), f(cg1), f(ca2), f(cg2)
    m_idx = np.asarray(m_idx, np.int64)
    n_idx = np.asarray(n_idx, np.int64)

    y = np.einsum("bci,oci->boi", x, w).astype(np.float32)  # [B, COUT, TOTAL]

    eh = np.zeros([IPAD, MN], np.float32)
    eh[np.arange(TOTAL), m_idx * MM + n_idx] = 1.0

    P1 = (ca1[:, None, :, None] * cg1[None, :, None, :]).reshape(MN, 1024)
    P2 = (ca2[:, None, :, None] * cg2[None, :, None, :]).reshape(MN, 1024)
    p12 = np.ascontiguousarray(np.concatenate([P1, P2], 1), dtype=np.float32)

    U0 = np.zeros([IPAD, UW], np.float32)
    U0[:, 0:MN] = eh
    U0[:TOTAL, MN + 8:UW] = y.transpose(2, 0, 1).reshape(TOTAL, BO)  # bo=b*32+o

    in_maps = []
    for k in range(NCORES):
        Uk = U0.copy()
        Uk[:TOTAL, MN:MN + 4] = d1[4 * k:4 * k + 4, :].T
        Uk[:TOTAL, MN + 4:MN + 8] = d2[4 * k:4 * k + 4, :].T
        in_maps.append({"uu": Uk, "p12": p12})
    return in_maps


def kernel(x, weight, d1, d2, ca1, cg1, ca2, cg2, m_idx, n_idx,
           _trace=False, _tmpdir=None):
    from concourse.bass_utils import run_bass_kernel_spmd

    nc = _build_bass()
    in_maps = _prep_inputs(x, weight, d1, d2, ca1, cg1, ca2, cg2,
                           m_idx, n_idx)
    res = run_bass_kernel_spmd(nc, in_maps, core_ids=list(range(NCORES)),
                               trace=_trace, tmpdir=_tmpdir,
                               trace_cores=list(range(NCORES)) if _trace else None,
                               stitch_traces=False)
    kernel.last_results = res
    m = np.maximum.reduce([r["out"] for r in res.results])  # [128, 2]
    return m.T.reshape(BO).reshape(B, COUT).astype(np.float32)
